# revision 50
# baseline (speedup 1.0000x reference)
"""RWKV v4 block (nn_Block_15109694947416) on 8 TRN2 NeuronCores.

Strategy (v2):
- Data-parallel over B: core i processes batch i (B=8). No collectives.
- Channel-major [C, T] on-chip layout, T in 4 chunks of 512.
- LayerNorm gain/bias and the token-shift mixing (x*tm + shift(x)*(1-tm)) are
  folded into the weights: k = u8 @ (g*tm*Wk) + shift(u8) @ (g*(1-tm)*Wk)
  + const, where u = 16*(x-m)*rstd is the fp8-quantized normalized input and
  the shifted operand is the SAME tile at a one-column offset, consumed by a
  single DoubleRow fp8 matmul per (k-block, out-block). Constants ride the
  activation bias; 1/scales ride the activation scale (per-partition APs).
- fp8e4 (e4m3) + MatmulPerfMode.DoubleRow for Wk/Wv/Wr (folded pairs), Wo and
  fWr; bf16 for fWk/fWv (precision headroom).
- rstd = exp(-0.5*ln(var+eps)) and sigmoid via exp:
  sigmoid(q)*z = z / (den*(1+exp(-q))) so the whole kernel uses only the
  natural_log_exp activation table (no table swaps).
- WKV scan unstabilized in fp32 scan-state (exact for this regime), carried
  bf16 between chunks; elementwise in bf16 where precision allows (DVE
  2x/4x modes).
- Residual path (x, x2, out) stays fp32 end to end.
"""

import math
import numpy as np
import ml_dtypes

B, T, C = 8, 2048, 1024
TC = 512                 # time chunk
NCH = T // TC            # chunks (4)
CB = C // 128            # channel blocks (8)
FB = 4 * C // 128        # ffn hidden blocks (32)
EPS = 1e-5
SU = 16.0                # u-activation scale (u8 stores 16*u)
SV = 32.0                # v/y chain scale (t_v stores 32*v, y8 stores 32*y)
NROW = 16
CVW = 200                # cvall [128, 200]: 8*16 tm rows, 32*2 ffn, ones

_CACHE = {}

# per-(cb) const rows: cvall col = cb*NROW + row
(EW, EU, CK, CV32, CRN, U1INIT, U2INIT, FTMK, FTMR,
 SCK, SCV, SCRN, SCO, SCFR, EPSR, LNSCR) = range(NROW)
# per-(ffn co) rows: col = 128 + co*2 + row
CFK, CFRN = range(2)
ONES_COL = 192           # 1.0 f32 (bitcast f32r for ones-matmul lhsT)
SCFK_COL = 193           # sqrt(8)/(SU*Sfk) fWk psum unscale incl kk8 fold
SCFV_COL = 194           # 1/(8*Sfv) fWv psum unscale
# carries tile [128, CB, 4] bf16 rows
CAR_U, CAR_U2, CAR_A, CAR_B = range(4)


def _bcast_free(ap, n):
    """[128,1] AP -> [128,n] stride-0 broadcast along free dim."""
    import concourse.bass as bass
    return bass.AP(tensor=ap.tensor, offset=ap.offset, ap=[ap.ap[0], [0, n]])


def _bcast_mid(ap, nmid):
    """[128,N] AP -> [128,nmid,N] stride-0 broadcast of a middle dim."""
    import concourse.bass as bass
    return bass.AP(tensor=ap.tensor, offset=ap.offset,
                   ap=[ap.ap[0], [0, nmid], ap.ap[1]])


def _pair_shift(t, a, n):
    """u-tile [128, CB, n+1] -> [128, 2, n] AP at block a: [p, i, t] =
    u[p, a, i + t]  (i=0: shifted/prev token, i=1: current token)."""
    import concourse.bass as bass
    ap = t[:, a, :]
    return bass.AP(tensor=ap.tensor, offset=ap.offset,
                   ap=[ap.ap[0], [1, 2], [1, n]])


def _build():
    import concourse.bass as bass
    import concourse.bacc as bacc
    import concourse.tile as tile
    import contextlib
    from concourse import mybir

    f32 = mybir.dt.float32
    f32r = mybir.dt.float32r
    bf16 = mybir.dt.bfloat16
    fp8 = mybir.dt.float8e4
    AF = mybir.ActivationFunctionType
    OP = mybir.AluOpType
    DR = mybir.MatmulPerfMode.DoubleRow

    nc = bacc.Bacc(None, target_bir_lowering=False, debug=False)

    xT = nc.dram_tensor("xT", [C, T], f32r, kind="ExternalInput")
    cvd = nc.dram_tensor("cvall", [128, CVW], f32r, kind="ExternalInput")
    ones16_in = nc.dram_tensor("ones128b", [128], bf16, kind="ExternalInput")
    ones_bin = nc.dram_tensor("onesb", [128], f32r, kind="ExternalInput")
    Wk2 = nc.dram_tensor("Wk2", [C, 2, C], fp8, kind="ExternalInput")
    Wv2 = nc.dram_tensor("Wv2", [C, 2, C], fp8, kind="ExternalInput")
    Wr2 = nc.dram_tensor("Wr2", [C, 2, C], fp8, kind="ExternalInput")
    Wo8 = nc.dram_tensor("Wo8", [C, C], fp8, kind="ExternalInput")
    fWkF = nc.dram_tensor("fWkF", [C, 2, 4 * C], fp8, kind="ExternalInput")
    fWrF = nc.dram_tensor("fWrF", [C, 2, C], fp8, kind="ExternalInput")
    fWvH = nc.dram_tensor("fWvH", [4 * C, C], fp8, kind="ExternalInput")
    fWvL = nc.dram_tensor("fWvL", [4 * C, C], fp8, kind="ExternalInput")
    outT = nc.dram_tensor("outT", [C, T], f32, kind="ExternalOutput")

    xTr = xT.rearrange("(cb p) t -> p cb t", p=128)
    outTr = outT.rearrange("(cb p) t -> p cb t", p=128)

    with tile.TileContext(nc) as tc:
      with contextlib.ExitStack() as ctx:
        consts = ctx.enter_context(tc.tile_pool(name="consts", bufs=1))
        dramp = ctx.enter_context(tc.tile_pool(name="dram", bufs=1, space="DRAM"))

        cvt = consts.tile([128, CVW], f32r)
        nc.sync.dma_start(out=cvt, in_=cvd[:, :])
        cvtf = cvt.bitcast(f32)
        ones_k16 = consts.tile([128, 1], bf16)
        nc.sync.dma_start(out=ones_k16, in_=ones16_in.rearrange("(p o) -> p o", o=1))
        ones_b16 = consts.tile([1, 128], bf16)
        nc.sync.dma_start(out=ones_b16, in_=ones16_in.rearrange("(o p) -> o p", o=1))
        rows2 = consts.tile([1, 2, T], bf16)
        ones_k = cvt[:, ONES_COL:ONES_COL + 1]

        def cva(cb, row):
            i = cb * NROW + row
            return cvtf[:, i:i + 1]

        def cvf(co, row):
            i = 128 + co * 2 + row
            return cvtf[:, i:i + 1]

        car = consts.tile([128, CB, 4], bf16)
        nc.vector.tensor_copy(out=car[:, :, CAR_U:CAR_U + 1],
                              in_=cvtf[:, 0:128].rearrange(
                                  "p (cb r) -> p cb r", r=NROW)[:, :, U1INIT:U1INIT + 1])
        nc.vector.tensor_copy(out=car[:, :, CAR_U2:CAR_U2 + 1],
                              in_=cvtf[:, 0:128].rearrange(
                                  "p (cb r) -> p cb r", r=NROW)[:, :, U2INIT:U2INIT + 1])
        nc.vector.memset(car[:, :, CAR_A:CAR_B + 1], 0.0)

        x2d = dramp.tile([NCH, 128, CB, TC], f32r, tag="x2d")
        kkd = dramp.tile([NCH, 128, FB, TC], fp8, tag="kkd")
        rrd = dramp.tile([NCH, 128, CB, TC], bf16, tag="rrd")

        def layernorm_stats(pools, x_t, sq16, rows, tmp):
            """Per-token mean + 16*rstd rows from x_t [128,CB,TC] f32.

            rows[:,0,:]=m  rows[:,1,:]=16*rstd (f32r-typed for the broadcast
            matmuls); rstd = exp(-0.5*ln(var+eps)+ln(16)) stays on the exp/ln
            activation table.
            """
            ps_stx, ps_stq = pools
            nc.scalar.activation(out=sq16, in_=x_t.bitcast(f32), func=AF.Square)
            for cb in range(CB):
                nc.tensor.matmul(ps_stx, ones_k, x_t[:, cb, :],
                                 start=(cb == 0), stop=(cb == CB - 1))
            for cb in range(CB):
                nc.tensor.matmul(ps_stq, ones_k16, sq16[:, cb, :],
                                 start=(cb == 0), stop=(cb == CB - 1))
            nc.vector.tensor_scalar_mul(rows[:, 0, :], ps_stx, 1.0 / C)
            nc.vector.tensor_mul(tmp[:, 0, :], rows[:, 0, :], rows[:, 0, :])
            nc.vector.scalar_tensor_tensor(
                out=tmp[:, 1, :], in0=ps_stq, scalar=1.0 / C,
                in1=tmp[:, 0, :], op0=OP.mult, op1=OP.subtract)
            nc.scalar.activation(out=tmp[:, 0, :], in_=tmp[:, 1, :],
                                 func=AF.Ln, bias=cvtf[0:1, EPSR + 0:EPSR + 1])
            nc.scalar.activation(out=tmp[:, 1, :], in_=tmp[:, 0, :],
                                 func=AF.Exp, scale=-0.5,
                                 bias=cvtf[0:1, LNSCR:LNSCR + 1])
            nc.vector.tensor_copy(out=rows[:, 1, :], in_=tmp[:, 1, :])

        # ================= Phase 1: fused time-mix =================
        with contextlib.ExitStack() as p1:
            wpool = p1.enter_context(tc.tile_pool(name="w1", bufs=1))
            dbl = p1.enter_context(tc.tile_pool(name="dbl1", bufs=2))
            sgl = p1.enter_context(tc.tile_pool(name="sgl1", bufs=1))
            rowp = p1.enter_context(tc.tile_pool(name="rows1", bufs=1))
            ps_mm = p1.enter_context(tc.tile_pool(name="ps_mm", bufs=4, space="PSUM"))
            ps_st = p1.enter_context(tc.tile_pool(name="ps_st", bufs=1, space="PSUM"))
            ps_bc = p1.enter_context(tc.tile_pool(name="ps_bc", bufs=1, space="PSUM"))

            def prep1(ic):
                """DMA + LN + u8p input prep for chunk ic (pipelined ahead)."""
                t0 = ic * TC
                x_t = dbl.tile([128, CB, TC], f32r, tag="x")
                nc.sync.dma_start(out=x_t, in_=xTr[:, :, t0:t0 + TC])
                x_f = x_t.bitcast(f32)
                sq16 = sgl.tile([128, CB, TC], bf16, tag="sq")
                rows = rowp.tile([1, 2, TC], bf16, tag="rows")
                tmp = rowp.tile([1, 2, TC], f32, tag="rtmp")
                ps_stx = ps_st.tile([1, TC], f32, tag="stx")
                ps_stq = ps_st.tile([1, TC], f32, tag="stq")
                layernorm_stats((ps_stx, ps_stq), x_t, sq16, rows, tmp)
                bc = ps_bc.tile([128, 2, TC], f32, tag="bc")
                nc.tensor.matmul(bc[:, 0, :], ones_b16, rows[:, 0, :])
                nc.tensor.matmul(bc[:, 1, :], ones_b16, rows[:, 1, :])
                s1 = sgl.tile([128, CB, TC], f32, tag="s1")
                nc.vector.tensor_sub(s1, x_f, _bcast_mid(bc[:, 0, :], CB))
                # u8p[:, cb, 1, :] = current token (16*u); [:, cb, 0, :] = prev
                u8p = dbl.tile([128, CB, 2, TC], fp8, tag="u8")
                nc.vector.tensor_mul(u8p[:, :, 1, :], s1,
                                     _bcast_mid(bc[:, 1, :], CB))
                nc.vector.tensor_copy(out=u8p[:, :, 0, 0:1],
                                      in_=car[:, :, CAR_U:CAR_U + 1])
                nc.vector.tensor_copy(out=u8p[:, :, 0, 1:TC],
                                      in_=u8p[:, :, 1, 0:TC - 1])
                nc.vector.tensor_copy(out=car[:, :, CAR_U:CAR_U + 1],
                                      in_=u8p[:, :, 1, TC - 1:TC])
                return x_t, u8p

            state = prep1(0)

            wk_t = wpool.tile([128, CB, 2, C], fp8, tag="wk")
            wv_t = wpool.tile([128, CB, 2, C], fp8, tag="wv")
            wr_t = wpool.tile([128, CB, 2, C], fp8, tag="wr")
            wo_t = wpool.tile([128, CB, C], fp8, tag="wo")
            for wt, wd in ((wk_t, Wk2), (wv_t, Wv2), (wr_t, Wr2)):
                wre = wd.rearrange("(a p) i m -> p a i m", p=128)
                for h in range(2):
                    nc.sync.dma_start(out=wt[:, 4 * h:4 * h + 4], in_=wre[:, 4 * h:4 * h + 4])
            nc.sync.dma_start(out=wo_t, in_=Wo8.rearrange("(a p) m -> p a m", p=128))

            for ic in range(NCH):
                x_t, u8p = state
                if ic + 1 < NCH:
                    state = prep1(ic + 1)
                ek = dbl.tile([128, CB, TC], bf16, tag="ek")
                ekv = dbl.tile([128, CB, TC], bf16, tag="ekv")
                e_r = sgl.tile([128, CB, TC], bf16, tag="er")
                for wt, post in (
                    (wk_t, lambda co, ps: nc.scalar.activation(
                        out=ek[:, co, :], in_=ps, func=AF.Exp,
                        scale=cva(0, SCK), bias=cva(co, CK))),
                    (wv_t, lambda co, ps: nc.scalar.activation(
                        out=ekv[:, co, :], in_=ps, func=AF.Identity,
                        scale=cva(0, SCV), bias=cva(co, CV32))),
                    (wr_t, lambda co, ps: nc.scalar.activation(
                        out=e_r[:, co, :], in_=ps, func=AF.Exp,
                        scale=cva(0, SCRN), bias=cva(co, CRN))),
                ):
                    for co in range(CB):
                        ps = ps_mm.tile([128, TC], f32, tag="mm")
                        csl = slice(co * 128, (co + 1) * 128)
                        for a in range(CB):
                            nc.tensor.matmul(ps, wt[:, a, :, csl],
                                             u8p[:, a, :, :],
                                             start=(a == 0), stop=(a == CB - 1),
                                             perf_mode=DR)
                        post(co, ps)

                # ekv = (32*v) * exp(k)   (in place)
                nc.vector.tensor_mul(ekv, ekv, ek)

                AB = sgl.tile([128, 2, CB, TC + 1], bf16, tag="AB")
                nc.vector.tensor_copy(out=AB[:, 0, :, 0:1], in_=car[:, :, CAR_A:CAR_A + 1])
                nc.vector.tensor_copy(out=AB[:, 1, :, 0:1], in_=car[:, :, CAR_B:CAR_B + 1])
                for cb in range(CB):
                    ew_b = _bcast_free(cva(cb, EW), TC)
                    nc.vector.tensor_tensor_scan(
                        out=AB[:, 0, cb, 1:TC + 1], data0=ew_b, data1=ekv[:, cb, :],
                        initial=AB[:, 0, cb, 0:1], op0=OP.mult, op1=OP.add)
                    nc.vector.tensor_tensor_scan(
                        out=AB[:, 1, cb, 1:TC + 1], data0=ew_b, data1=ek[:, cb, :],
                        initial=AB[:, 1, cb, 0:1], op0=OP.mult, op1=OP.add)
                nc.vector.tensor_copy(out=car[:, :, CAR_A:CAR_A + 1],
                                      in_=AB[:, 0, :, TC:TC + 1])
                nc.vector.tensor_copy(out=car[:, :, CAR_B:CAR_B + 1],
                                      in_=AB[:, 1, :, TC:TC + 1])

                # num -> ekv, den -> ek (in place)
                for cb in range(CB):
                    eu_s = cva(cb, EU)
                    nc.vector.scalar_tensor_tensor(
                        out=ekv[:, cb, :], in0=ekv[:, cb, :], scalar=eu_s,
                        in1=AB[:, 0, cb, 0:TC], op0=OP.mult, op1=OP.add)
                    nc.vector.scalar_tensor_tensor(
                        out=ek[:, cb, :], in0=ek[:, cb, :], scalar=eu_s,
                        in1=AB[:, 1, cb, 0:TC], op0=OP.mult, op1=OP.add)
                # den2 = den * (1 + e_r): folds the r-sigmoid into the division
                den2 = sgl.tile([128, CB, TC], f32, tag="s1")
                nc.vector.scalar_tensor_tensor(
                    out=den2, in0=e_r, scalar=1.0, in1=ek,
                    op0=OP.add, op1=OP.mult)
                nc.vector.reciprocal_approx_fast(out=den2, in_=den2)
                y8 = sgl.tile([128, CB, TC], fp8, tag="er")
                nc.gpsimd.tensor_mul(y8, ekv, den2)

                x2 = sgl.tile([128, CB, TC], f32r, tag="s1")
                att = sgl.tile([128, CB, TC], bf16, tag="sq")
                x_f = x_t.bitcast(f32)
                for co in range(CB):
                    ps = ps_mm.tile([128, TC], f32, tag="mm")
                    csl = slice(co * 128, (co + 1) * 128)
                    for j in range(CB // 2):
                        nc.tensor.matmul(ps, wo_t[:, 2 * j:2 * j + 2, csl],
                                         y8[:, 2 * j:2 * j + 2, :],
                                         start=(j == 0), stop=(j == CB // 2 - 1),
                                         perf_mode=DR)
                    nc.scalar.activation(out=att[:, co, :], in_=ps,
                                         func=AF.Identity, scale=cva(0, SCO))
                nc.vector.tensor_add(x2, x_f, att)
                nc.sync.dma_start(out=x2d[ic], in_=x2)

                # LN2 stats for this chunk (x2 already in SBUF) -> rows2
                t0 = ic * TC
                sq2 = sgl.tile([128, CB, TC], bf16, tag="sq")
                nc.scalar.activation(out=sq2, in_=x2.bitcast(f32), func=AF.Square)
                ps_sx = ps_st.tile([1, TC], f32, tag="stx")
                ps_sq = ps_st.tile([1, TC], f32, tag="stq")
                for cb in range(CB):
                    nc.tensor.matmul(ps_sx, ones_k, x2[:, cb, :],
                                     start=(cb == 0), stop=(cb == CB - 1))
                for cb in range(CB):
                    nc.tensor.matmul(ps_sq, ones_k16, sq2[:, cb, :],
                                     start=(cb == 0), stop=(cb == CB - 1))
                tmp = rowp.tile([1, 2, TC], f32, tag="rtmp")
                nc.vector.tensor_scalar_mul(rows2[:, 0, t0:t0 + TC], ps_sx, 1.0 / C)
                nc.vector.tensor_mul(tmp[:, 0, :], rows2[:, 0, t0:t0 + TC],
                                     rows2[:, 0, t0:t0 + TC])
                nc.vector.scalar_tensor_tensor(
                    out=tmp[:, 1, :], in0=ps_sq, scalar=1.0 / C,
                    in1=tmp[:, 0, :], op0=OP.mult, op1=OP.subtract)
                nc.scalar.activation(out=tmp[:, 0, :], in_=tmp[:, 1, :],
                                     func=AF.Ln, bias=cvtf[0:1, EPSR + 0:EPSR + 1])
                nc.scalar.activation(out=tmp[:, 1, :], in_=tmp[:, 0, :],
                                     func=AF.Exp, scale=-0.5,
                                     bias=cvtf[0:1, LNSCR:LNSCR + 1])
                nc.vector.tensor_copy(out=rows2[:, 1, t0:t0 + TC], in_=tmp[:, 1, :])

        # ================= Phase 2a: FFN up (kk + rr) =================
        with contextlib.ExitStack() as p2:
            wpool = p2.enter_context(tc.tile_pool(name="w2", bufs=1))
            dbl = p2.enter_context(tc.tile_pool(name="dbl2", bufs=2))
            sgl = p2.enter_context(tc.tile_pool(name="sgl2", bufs=1))
            ps_mm = p2.enter_context(tc.tile_pool(name="ps_mm2", bufs=4, space="PSUM"))
            ps_bc = p2.enter_context(tc.tile_pool(name="ps_bc2", bufs=1, space="PSUM"))

            def prep2(ic):
                t0 = ic * TC
                x2_t = dbl.tile([128, CB, TC], f32r, tag="x2i")
                nc.sync.dma_start(out=x2_t, in_=x2d[ic])
                x2_f = x2_t.bitcast(f32)
                bc = ps_bc.tile([128, 2, TC], f32, tag="bc")
                nc.tensor.matmul(bc[:, 0, :], ones_b16, rows2[:, 0, t0:t0 + TC])
                nc.tensor.matmul(bc[:, 1, :], ones_b16, rows2[:, 1, t0:t0 + TC])
                s1 = sgl.tile([128, CB, TC], f32, tag="s12")
                nc.vector.tensor_sub(s1, x2_f, _bcast_mid(bc[:, 0, :], CB))
                u2p = dbl.tile([128, CB, 2, TC], fp8, tag="u2")
                nc.vector.tensor_mul(u2p[:, :, 1, :], s1,
                                     _bcast_mid(bc[:, 1, :], CB))
                nc.vector.tensor_copy(out=u2p[:, :, 0, 0:1],
                                      in_=car[:, :, CAR_U2:CAR_U2 + 1])
                nc.vector.tensor_copy(out=u2p[:, :, 0, 1:TC],
                                      in_=u2p[:, :, 1, 0:TC - 1])
                nc.vector.tensor_copy(out=car[:, :, CAR_U2:CAR_U2 + 1],
                                      in_=u2p[:, :, 1, TC - 1:TC])
                return (u2p,)

            state = prep2(0)

            fwk_t = wpool.tile([128, 2 * CB, 4 * C], fp8, tag="fwk")
            fwkr = fWkF.rearrange("(a p) i m -> p a i m", p=128)
            for h in range(2):
                hs = slice(h * 2 * C, (h + 1) * 2 * C)
                for a in range(CB):
                    nc.sync.dma_start(out=fwk_t[:, 2 * a:2 * a + 2, hs],
                                      in_=fwkr[:, a, :, hs])
            fwr_t = wpool.tile([128, 2 * CB, C], fp8, tag="fwr")
            fwrr = fWrF.rearrange("(a p) i m -> p a i m", p=128)
            for a in range(CB):
                nc.sync.dma_start(out=fwr_t[:, 2 * a:2 * a + 2, :],
                                  in_=fwrr[:, a, :, :])

            for ic in range(NCH):
                (u2p,) = state
                if ic + 1 < NCH:
                    state = prep2(ic + 1)
                for q in range(4):
                    rt = sgl.tile([128, CB, TC], bf16, tag="rt")
                    for j in range(CB):
                        co = q * CB + j
                        ps = ps_mm.tile([128, TC], f32, tag="mm")
                        csl = slice(co * 128, (co + 1) * 128)
                        for a in range(CB):
                            nc.tensor.matmul(ps, fwk_t[:, 2 * a:2 * a + 2, csl],
                                             u2p[:, a, :, :],
                                             start=(a == 0), stop=(a == CB - 1),
                                             perf_mode=DR)
                        nc.scalar.activation(out=rt[:, j, :], in_=ps, func=AF.Relu,
                                             scale=cvtf[:, SCFK_COL:SCFK_COL + 1],
                                             bias=cvf(co, CFK))
                    kkq = dbl.tile([128, CB, TC], fp8, tag="kkq")
                    # kk8 = (sqrt(8)*rt)^2 = 8*relu(k2)^2, quantized e4m3
                    nc.scalar.activation(out=kkq, in_=rt, func=AF.Square,
                                         scale=2.8284271247461903)
                    nc.sync.dma_start(out=kkd[ic][:, q * CB:(q + 1) * CB, :], in_=kkq)

                e_rr = sgl.tile([128, CB, TC], bf16, tag="rt")
                for co in range(CB):
                    ps = ps_mm.tile([128, TC], f32, tag="mm")
                    csl = slice(co * 128, (co + 1) * 128)
                    for a in range(CB):
                        nc.tensor.matmul(ps, fwr_t[:, 2 * a:2 * a + 2, csl],
                                         u2p[:, a, :, :],
                                         start=(a == 0), stop=(a == CB - 1),
                                         perf_mode=DR)
                    nc.scalar.activation(out=e_rr[:, co, :], in_=ps, func=AF.Exp,
                                         scale=cva(0, SCFR),
                                         bias=cvf(co, CFRN))

                den = sgl.tile([128, CB, TC], f32, tag="s12")
                nc.vector.tensor_scalar_add(out=den, in0=e_rr, scalar1=1.0)
                nc.vector.reciprocal_approx_fast(out=den, in_=den)
                rr16 = sgl.tile([128, CB, TC], bf16, tag="rr")
                nc.vector.tensor_copy(out=rr16, in_=den)
                nc.sync.dma_start(out=rrd[ic], in_=rr16)

        # ================= Phase 2b: FFN down + residual =================
        with contextlib.ExitStack() as p3:
            wpool = p3.enter_context(tc.tile_pool(name="w3", bufs=1))
            dbl = p3.enter_context(tc.tile_pool(name="dbl3", bufs=2))
            halfp = p3.enter_context(tc.tile_pool(name="half3", bufs=4))
            ps_mm = p3.enter_context(tc.tile_pool(name="ps_mm3", bufs=4, space="PSUM"))

            def prep3_kk(ic):
                kk0 = halfp.tile([128, FB // 2, TC], fp8, tag="kkh")
                nc.sync.dma_start(out=kk0, in_=kkd[ic][:, 0:FB // 2, :])
                kk1 = halfp.tile([128, FB // 2, TC], fp8, tag="kkh")
                nc.sync.dma_start(out=kk1, in_=kkd[ic][:, FB // 2:FB, :])
                return kk0, kk1

            def prep3_rx(ic):
                rr16 = dbl.tile([128, CB, TC], bf16, tag="rri")
                nc.sync.dma_start(out=rr16, in_=rrd[ic])
                x2_t = dbl.tile([128, CB, TC], f32, tag="x2b")
                nc.sync.dma_start(out=x2_t, in_=x2d[ic].bitcast(f32))
                return rr16, x2_t

            kkstate = prep3_kk(0)

            fwvh_t = wpool.tile([128, FB, C], fp8, tag="fwvh")
            fwvhr = fWvH.rearrange("(a p) m -> p a m", p=128)
            fwvl_t = wpool.tile([128, FB, C], fp8, tag="fwvl")
            fwvlr = fWvL.rearrange("(a p) m -> p a m", p=128)
            for s in range(4):
                nc.sync.dma_start(out=fwvh_t[:, s * 8:(s + 1) * 8, :],
                                  in_=fwvhr[:, s * 8:(s + 1) * 8, :])
            for s in range(4):
                nc.sync.dma_start(out=fwvl_t[:, s * 8:(s + 1) * 8, :],
                                  in_=fwvlr[:, s * 8:(s + 1) * 8, :])
            rxstate = prep3_rx(0)

            for ic in range(NCH):
                t0 = ic * TC
                kk0, kk1 = kkstate
                rr16, x2_t = rxstate
                if ic + 1 < NCH:
                    kkstate = prep3_kk(ic + 1)
                    rxstate = prep3_rx(ic + 1)

                out_t = dbl.tile([128, CB, TC], f32, tag="out")
                for co in range(CB):
                    ps = ps_mm.tile([128, TC], f32, tag="mm")
                    csl = slice(co * 128, (co + 1) * 128)
                    for wi, w_t in enumerate((fwvh_t, fwvl_t)):
                        for j in range(FB // 2):
                            kkh = kk0 if 2 * j < FB // 2 else kk1
                            jj = (2 * j) % (FB // 2)
                            nc.tensor.matmul(ps, w_t[:, 2 * j:2 * j + 2, csl],
                                             kkh[:, jj:jj + 2, :],
                                             start=(wi == 0 and j == 0),
                                             stop=(wi == 1 and j == FB // 2 - 1),
                                             perf_mode=DR)
                    # out = x2 + rr * ps/(8*Sfv)
                    nc.vector.scalar_tensor_tensor(
                        out=ps, in0=ps, scalar=cvtf[:, SCFV_COL:SCFV_COL + 1],
                        in1=rr16[:, co, :], op0=OP.mult, op1=OP.mult)
                    nc.vector.tensor_add(out_t[:, co, :], x2_t[:, co, :], ps)
                nc.sync.dma_start(out=outTr[:, :, t0:t0 + TC], in_=out_t)

    nc.finalize()
    return nc


def _prep_maps(inputs):
    E4 = ml_dtypes.float8_e4m3
    BF = ml_dtypes.bfloat16
    f32 = np.float32

    x = np.asarray(inputs["x"], f32)
    ln1_g = np.asarray(inputs["ln1_g"], f32)
    ln1_b = np.asarray(inputs["ln1_b"], f32)
    ln2_g = np.asarray(inputs["ln2_g"], f32)
    ln2_b = np.asarray(inputs["ln2_b"], f32)
    tmk = np.asarray(inputs["tmk"], f32)
    tmv = np.asarray(inputs["tmv"], f32)
    tmr = np.asarray(inputs["tmr"], f32)
    ftmk = np.asarray(inputs["ftmk"], f32)
    ftmr = np.asarray(inputs["ftmr"], f32)
    ew = np.exp(-np.exp(np.asarray(inputs["time_decay"], f32))).astype(f32)
    eu = np.exp(np.asarray(inputs["time_first"], f32)).astype(f32)

    def pow2s(m, target=224.0):
        m = float(m)
        if m <= 0:
            return 1.0
        return 2.0 ** math.floor(math.log2(target / m))

    def fold2(W, tm, g, b):
        """Pack [(1-tm)*g*W | tm*g*W] * S as [C, 2, Co] e4m3; const = b@W."""
        W = np.asarray(W, f32)
        Wa = ((1.0 - tm) * g)[:, None] * W
        Wb = (tm * g)[:, None] * W
        S = pow2s(max(np.abs(Wa).max(), np.abs(Wb).max()))
        P = np.stack([Wa, Wb], axis=1) * S
        return np.ascontiguousarray(P).astype(E4), S, (b @ W).astype(f32)

    Wk2, Sk, ck = fold2(inputs["Wk"], tmk, ln1_g, ln1_b)
    Wv2, Sv, cv_ = fold2(inputs["Wv"], tmv, ln1_g, ln1_b)
    Wr2, Sr, cr_ = fold2(inputs["Wr"], tmr, ln1_g, ln1_b)

    Wo = np.asarray(inputs["Wo"], f32)
    So = pow2s(np.abs(Wo).max())
    Wo8 = (Wo * So).astype(E4)

    fWkF, Sfk, cfk = fold2(inputs["fWk"], ftmk, ln2_g, ln2_b)
    fWrF, Sfr, cfr = fold2(inputs["fWr"], ftmr, ln2_g, ln2_b)

    fWv = np.asarray(inputs["fWv"], f32)
    Sfv = pow2s(np.abs(fWv).max())
    fWvH = (fWv * Sfv).astype(E4)
    fWvL = (fWv * Sfv - fWvH.astype(f32)).astype(E4)

    def plane(vec):
        # [C] indexed by channel -> [128, CB] (p, cb)
        return np.ascontiguousarray(np.asarray(vec, f32).reshape(CB, 128).T)

    with np.errstate(divide="ignore", invalid="ignore"):
        u1i = np.where(ln1_g != 0, -SU * ln1_b / ln1_g, 0.0)
        u2i = np.where(ln2_g != 0, -SU * ln2_b / ln2_g, 0.0)
    u1i = np.clip(np.nan_to_num(u1i), -400, 400)
    u2i = np.clip(np.nan_to_num(u2i), -400, 400)

    rows = np.zeros((128, CB, NROW), f32)
    rows[:, :, EW] = plane(ew)
    rows[:, :, EU] = plane(eu)
    rows[:, :, CK] = plane(ck)
    rows[:, :, CV32] = plane(SV * cv_)
    rows[:, :, CRN] = plane(-cr_)
    rows[:, :, U1INIT] = plane(u1i)
    rows[:, :, U2INIT] = plane(u2i)
    rows[:, :, FTMK] = plane(ftmk)
    rows[:, :, FTMR] = plane(ftmr)
    rows[:, :, SCK] = 1.0 / (SU * Sk)
    rows[:, :, SCV] = SV / (SU * Sv)
    rows[:, :, SCRN] = -1.0 / (SU * Sr)
    rows[:, :, SCO] = 1.0 / (SV * So)
    rows[:, :, SCFR] = -1.0 / (SU * Sfr)
    rows[:, :, EPSR] = EPS
    rows[:, :, LNSCR] = math.log(SU)

    cvall = np.zeros((128, CVW), f32)
    cvall[:, 0:128] = rows.reshape(128, 128)
    ffn = np.zeros((128, FB, 2), f32)
    ffn[:, :, CFK] = np.ascontiguousarray(cfk.reshape(FB, 128).T)
    ffn[:, 0:CB, CFRN] = np.ascontiguousarray((-cfr).reshape(CB, 128).T)
    cvall[:, 128:192] = ffn.reshape(128, 64)
    cvall[:, ONES_COL] = 1.0
    cvall[:, SCFK_COL] = 1.0 / (SU * Sfk)
    cvall[:, SCFV_COL] = 1.0 / (8.0 * Sfv)

    common = {
        "cvall": cvall,
        "ones128b": np.ones(128, BF), "onesb": np.ones(128, f32),
        "Wk2": Wk2, "Wv2": Wv2, "Wr2": Wr2, "Wo8": Wo8,
        "fWkF": fWkF, "fWrF": fWrF, "fWvH": fWvH, "fWvL": fWvL,
    }
    return [{**common, "xT": np.ascontiguousarray(x[b].T)} for b in range(B)]


def get_nc():
    if "nc" not in _CACHE:
        _CACHE["nc"] = _build()
    return _CACHE["nc"]


def kernel(**inputs):
    from concourse.bass_utils import run_bass_kernel_spmd
    nc = get_nc()
    in_maps = _prep_maps(inputs)
    res = run_bass_kernel_spmd(nc, in_maps, core_ids=list(range(B)))
    return np.stack([np.ascontiguousarray(r["outT"].T) for r in res.results])
